# revision 20
# baseline (speedup 1.0000x reference)
"""Trainium2 Bass kernel for Llama attention (B=2, S=2048, H=4096, 32 heads).

Sharding: tensor-parallel across heads over 8 NeuronCores. Each core owns
d_shard = 512 hidden dims (4 heads): Wq/Wk/Wv column-sharded, Wo row-sharded.
All cores see the full (transposed) hidden states; partial outputs are summed
on the host (the Wo row-parallel all-reduce).

v2: all matmul operands in bf16 (psum stays f32), batched DMA, LDWEIGHTS
thinning via loop reordering, causal-diagonal trimming in attention.

Per-core device program (one NEFF, three phases through DRAM intermediates):
  A) QKV projections. Per 1024-token group: one DMA for the hs slab,
     Wq/Wk streamed per (head, group) with k-outer/nl-inner matmuls so each
     stationary tile is loaded once per two PSUM tiles; RoPE fused into the
     PSUM evacuation; Wv resident. Q^T,K^T land in [d, tok] bf16, V in
     [tok, d] bf16.
  B) Causal attention per (batch, head) in the transposed score layout
     S^T[k, q] so softmax needs no transposes: exp (no max-subtraction),
     diagonal 128-blocks restricted to the live q-range, row-sums via a
     ones-vector matmul, 1/L via Ln/Exp on ScalarE, unnormalized O^T
     accumulation, normalized on PSUM evacuation. O^T stored bf16.
  C) out = O^T.T @ Wo accumulated over the 4 local heads, jd-outer so each
     O^T stationary tile is loaded once per 4 moving tiles; Wo resident
     bf16; outputs staged in [128, 2048] slabs to cut DMA descriptor count.
"""

import math
import os
import sys

import numpy as np

for _p in ("/root/.axon_site/_ro/trn_rl_repo", "/opt/trn_rl_repo"):
    if os.path.isdir(_p) and _p not in sys.path:
        sys.path.append(_p)

import concourse.bass as bass
import concourse.mybir as mybir
import concourse.tile as tile
from concourse import bacc
from concourse import bass_utils

# Bind Exp AND Ln to the one table set containing both
# (natural_log_exp_and_others). The default chooser binds Exp to
# exp_and_others and Ln to natural_log, which makes the ACT stream reload
# table sets (~2.7us each) around every softmax-denominator Ln. Indices into
# act_info.json must be preserved, so only set CONTENTS are edited.
_orig_get_act_tables = bacc.get_activation_tables


def _patched_get_act_tables(arch):
    tabs = {k: set(v) for k, v in _orig_get_act_tables(arch).items()}
    AF = mybir.ActivationFunctionType
    if "natural_log_exp_and_others" in tabs:
        combined = tabs["natural_log_exp_and_others"]
        if AF.Exp in combined and AF.Ln in combined:
            for name, fns in tabs.items():
                if name != "natural_log_exp_and_others":
                    fns.discard(AF.Exp)
                    fns.discard(AF.Ln)
    return tabs


bacc.get_activation_tables = _patched_get_act_tables

F32 = mybir.dt.float32
F32R = mybir.dt.float32r
BF16 = mybir.dt.bfloat16

HIDDEN = 4096
NUM_HEADS = 32
HEAD_DIM = 128
ROPE_BASE = 10000.0
N_CORES = 8


class Cfg:
    def __init__(self, hidden=HIDDEN, d_shard=HIDDEN // N_CORES, s_batch=2048,
                 n_batch=2, tokt=512, tgrp=2):
        self.hidden = hidden
        self.d_shard = d_shard
        self.s_batch = s_batch
        self.n_batch = n_batch
        self.tokt = tokt          # token tile (psum free dim)
        self.tgrp = tgrp          # token tiles per phase-A group
        self.KC = hidden // 128   # contraction chunks
        self.HL = d_shard // HEAD_DIM   # local heads
        self.NTOK = n_batch * s_batch
        self.NT = self.NTOK // tokt
        assert self.NT % tgrp == 0
        self.NG = self.NT // tgrp
        assert s_batch % tokt == 0  # a token tile never straddles batches
        self.KCPB = s_batch // 128      # key chunks per batch
        self.QTPB = s_batch // tokt     # q tiles per batch
        self.NDIAG = tokt // 128        # diagonal 128-blocks per q tile
        self.scale = HEAD_DIM ** -0.5


def build_nc(cfg: Cfg, n_cores=N_CORES, phases="ABC"):
    c = cfg
    nc = bacc.Bacc("TRN2", target_bir_lowering=False, debug=False,
                   num_devices=n_cores)
    hsT = nc.dram_tensor("hsT", [c.hidden, c.NTOK], BF16, kind="ExternalInput")
    Wq = nc.dram_tensor("Wq", [c.hidden, c.d_shard], BF16, kind="ExternalInput")
    Wk = nc.dram_tensor("Wk", [c.hidden, c.d_shard], BF16, kind="ExternalInput")
    Wv = nc.dram_tensor("Wv", [c.hidden, c.d_shard], BF16, kind="ExternalInput")
    Wo = nc.dram_tensor("Wo", [c.d_shard, c.hidden], BF16, kind="ExternalInput")
    cosT = nc.dram_tensor("cosT", [128, c.s_batch], BF16, kind="ExternalInput")
    sinT = nc.dram_tensor("sinT", [128, c.s_batch], BF16, kind="ExternalInput")
    out = nc.dram_tensor("out", [c.NTOK, c.hidden], BF16, kind="ExternalOutput")
    QTd = nc.dram_tensor("QTd", [c.d_shard, c.NTOK], BF16, kind="Internal")
    KTd = nc.dram_tensor("KTd", [c.d_shard, c.NTOK], BF16, kind="Internal")
    Vd = nc.dram_tensor("Vd", [c.NTOK, c.d_shard], BF16, kind="Internal")
    OTd = nc.dram_tensor("OTd", [c.d_shard, c.NTOK], BF16, kind="Internal")

    AF = mybir.ActivationFunctionType
    gw = c.tokt * c.tgrp                  # group width (tokens)
    with tile.TileContext(nc) as tc:
        if True:
            # ---------------- Phase A: projections + RoPE ----------------
            if "A" in phases:
              _sidA, _ = nc.enter_named_scope("phA", False)
              with (tc.tile_pool(name="constA", bufs=1) as cpool,
                  tc.tile_pool(name="hs", bufs=2) as hsp,
                  tc.tile_pool(name="wqk", bufs=2) as wp,
                  tc.tile_pool(name="evA", bufs=3) as evp,
                  tc.tile_pool(name="psA", bufs=2, space="PSUM") as psA,
                  tc.tile_pool(name="psV", bufs=1, space="PSUM") as psV):
                # group-0 hs first so the PE can start before the consts land;
                # each slab in two k-halves so matmuls begin after half one.
                KH = c.KC // 2

                def load_hs(g):
                    gcol = g * gw
                    halves = []
                    for hf in range(2):
                        t = hsp.tile([128, KH * gw], BF16, tag=f"hs{hf}",
                                     name=f"hs_{g}_{hf}")
                        nc.sync.dma_start(
                            t[:],
                            hsT.ap()[hf * KH * 128:(hf + 1) * KH * 128,
                                     gcol:gcol + gw]
                            .rearrange("(c p) n -> p c n", p=128))
                        halves.append(t)
                    return halves

                hts0 = load_hs(0)
                cos_sb = cpool.tile([128, c.s_batch], BF16, tag="cos")
                nc.sync.dma_start(cos_sb[:], cosT.ap())
                sin_sb = cpool.tile([128, c.s_batch], BF16, tag="sin")
                nc.sync.dma_start(sin_sb[:], sinT.ap())
                # Wv resident: [128, KC*512] bf16 (4.2 MB)
                wv_res = cpool.tile([128, c.KC * c.d_shard], BF16, tag="wv")
                nc.sync.dma_start(
                    wv_res[:],
                    Wv.ap().rearrange("(c p) n -> p c n", p=128))
                for g in range(c.NG):
                    gcol = g * gw
                    hts = hts0 if g == 0 else load_hs(g)
                    for wdram, outdram in ((Wq, QTd), (Wk, KTd)):
                        for h in range(c.HL):
                            wt = wp.tile([128, c.KC * 128], BF16, tag="w")
                            nc.sync.dma_start(
                                wt[:],
                                wdram.ap()[:, h * 128:(h + 1) * 128]
                                .rearrange("(c p) n -> p c n", p=128))
                            pss = [psA.tile([128, c.tokt], F32, tag=f"psA{nl}",
                                            name=f"psA_{g}_{h}_{nl}")
                                   for nl in range(c.tgrp)]
                            # k-outer, nl-inner: one LDWEIGHTS per k chunk
                            for k in range(c.KC):
                                hk = hts[k // KH]
                                ko = (k % KH) * gw
                                for nl in range(c.tgrp):
                                    nc.tensor.matmul(
                                        pss[nl][:],
                                        wt[:, k * 128:(k + 1) * 128],
                                        hk[:, ko + nl * c.tokt:
                                           ko + (nl + 1) * c.tokt],
                                        start=(k == 0), stop=(k == c.KC - 1))
                            o = evp.tile([128, gw], BF16, tag="ev_o")
                            for nl in range(c.tgrp):
                                ps = pss[nl]
                                col0 = gcol + nl * c.tokt
                                lc = col0 % c.s_batch
                                a = evp.tile([128, c.tokt], F32, tag="ev_a",
                                             bufs=2)
                                bt = evp.tile([128, c.tokt], F32, tag="ev_b",
                                              bufs=2)
                                nc.vector.tensor_mul(
                                    a[:], ps[:], cos_sb[:, lc:lc + c.tokt])
                                nc.vector.tensor_mul(
                                    bt[0:64, :], ps[64:128, :],
                                    sin_sb[0:64, lc:lc + c.tokt])
                                nc.vector.tensor_mul(
                                    bt[64:128, :], ps[0:64, :],
                                    sin_sb[64:128, lc:lc + c.tokt])
                                nc.vector.tensor_add(
                                    o[:, nl * c.tokt:(nl + 1) * c.tokt],
                                    a[:], bt[:])
                            nc.sync.dma_start(
                                outdram.ap()[h * 128:(h + 1) * 128,
                                             gcol:gcol + gw], o[:])
                    # V = hs @ Wv in [tok, d] layout; k-outer over 4 psum banks
                    for nl in range(c.tgrp):
                        vps = [psV.tile([128, c.d_shard], F32, tag=f"v{i}",
                                        name=f"vps{g}_{nl}_{i}")
                               for i in range(c.tokt // 128)]
                        for k in range(c.KC):
                            hk = hts[k // KH]
                            ko = (k % KH) * gw
                            for i in range(c.tokt // 128):
                                t0 = ko + nl * c.tokt + i * 128
                                nc.tensor.matmul(
                                    vps[i][:], hk[:, t0:t0 + 128],
                                    wv_res[:, k * c.d_shard:
                                           (k + 1) * c.d_shard],
                                    start=(k == 0), stop=(k == c.KC - 1))
                        for i in range(c.tokt // 128):
                            ev = evp.tile([128, c.d_shard], BF16, tag="ev_v")
                            nc.scalar.copy(ev[:], vps[i][:])
                            r0 = gcol + nl * c.tokt + i * 128
                            nc.sync.dma_start(Vd.ap()[r0:r0 + 128, :], ev[:])

            # ---------------- Phase B: causal attention ----------------
            if "B" in phases:
              if "A" in phases:
                  nc.leave_named_scope("phA", _sidA, False)
              _sidB, _ = nc.enter_named_scope("phB", False)
              with (tc.tile_pool(name="constB", bufs=1) as cbp,
                  tc.tile_pool(name="qkv", bufs=3) as qkvp,
                  tc.tile_pool(name="pb", bufs=3) as pbp,
                  tc.tile_pool(name="sm", bufs=2) as smp,
                  tc.tile_pool(name="psS", bufs=4, space="PSUM") as psS,
                  tc.tile_pool(name="psO", bufs=2, space="PSUM") as psO,
                  tc.tile_pool(name="psL", bufs=2, space="PSUM") as psL):
                ones_f = cbp.tile([128, 1], F32, tag="ones_f")
                nc.vector.memset(ones_f[:], 1.0)
                ones_sb = cbp.tile([128, 1], BF16, tag="ones")
                nc.vector.tensor_copy(ones_sb[:], ones_f[:])
                # one triangular mask [128, tokt]: keep where col - part >= 0.
                # Diagonal block td of q-tile j uses mask[:, :tokt-128*td] on
                # the q-range starting at 128*td (q - k = col - part there).
                mf = cbp.tile([128, c.tokt], F32, tag="maskf")
                nc.gpsimd.memset(mf[:], 1.0)
                nc.gpsimd.affine_select(
                    out=mf[:], in_=mf[:], compare_op=mybir.AluOpType.is_ge,
                    fill=0.0, base=0, pattern=[[1, c.tokt]],
                    channel_multiplier=-1)
                mask = cbp.tile([128, c.tokt], BF16, tag="mask")
                nc.vector.tensor_copy(mask[:], mf[:])
                for b in range(c.n_batch):
                    for h in range(c.HL):
                        s0 = b * c.s_batch
                        qt = qkvp.tile([128, c.s_batch], BF16, tag="qt")
                        nc.sync.dma_start(
                            qt[:], QTd.ap()[h * 128:(h + 1) * 128,
                                            s0:s0 + c.s_batch])
                        kt = qkvp.tile([128, c.s_batch], BF16, tag="kt")
                        nc.sync.dma_start(
                            kt[:], KTd.ap()[h * 128:(h + 1) * 128,
                                            s0:s0 + c.s_batch])
                        vt = qkvp.tile([128, c.KCPB * 128], BF16, tag="vt")
                        nc.sync.dma_start(
                            vt[:],
                            Vd.ap()[s0:s0 + c.s_batch, h * 128:(h + 1) * 128]
                            .rearrange("(c p) n -> p c n", p=128))
                        for j in range(c.QTPB):
                            nchunks = (j + 1) * c.tokt // 128
                            ot_ps = psO.tile([128, c.tokt], F32, tag="ot")
                            l_ps = psL.tile([1, c.tokt], F32, tag="l")
                            prev = None

                            def flush(last):
                                p_, i_, q0_ = prev
                                qn = c.tokt - q0_
                                nc.tensor.matmul(
                                    ot_ps[:, q0_:], vt[:, i_ * 128:(i_ + 1) * 128],
                                    p_[:, :qn], start=(i_ == 0), stop=last)
                                nc.tensor.matmul(
                                    l_ps[:, q0_:], ones_sb[:], p_[:, :qn],
                                    start=(i_ == 0), stop=last)

                            for i in range(nchunks):
                                td = i - (j * c.tokt) // 128
                                # q-range [q0, tokt) of this tile is live
                                q0 = max(td, 0) * 128
                                qn = c.tokt - q0
                                s_ps = psS.tile([128, c.tokt], F32, tag="s")
                                nc.tensor.matmul(
                                    s_ps[:, :qn], kt[:, i * 128:(i + 1) * 128],
                                    qt[:, j * c.tokt + q0:(j + 1) * c.tokt],
                                    start=True, stop=True)
                                p = pbp.tile([128, c.tokt], BF16, tag="p")
                                nc.scalar.activation(p[:, :qn], s_ps[:, :qn],
                                                     AF.Exp, scale=c.scale)
                                if td >= 0:
                                    nc.vector.tensor_mul(p[:, :qn], p[:, :qn],
                                                         mask[:, :qn])
                                if prev is not None:
                                    flush(False)
                                prev = (p, i, q0)
                            flush(True)
                            lg = smp.tile([1, c.tokt], F32, tag="lg")
                            nc.scalar.activation(lg[:], l_ps[:], AF.Ln)
                            rc = smp.tile([1, c.tokt], F32, tag="rc")
                            nc.scalar.activation(rc[:], lg[:], AF.Exp,
                                                 scale=-1.0)
                            bc = smp.tile([128, c.tokt], F32, tag="bc")
                            nc.gpsimd.partition_broadcast(bc[:], rc[:])
                            q0 = s0 + j * c.tokt
                            otn = smp.tile([128, c.tokt], BF16, tag="otn",
                                           bufs=3)
                            nc.vector.tensor_mul(otn[:], ot_ps[:], bc[:])
                            nc.sync.dma_start(
                                OTd.ap()[h * 128:(h + 1) * 128,
                                         q0:q0 + c.tokt], otn[:])

            # ---------------- Phase C: output projection ----------------
            if "C" in phases:
              if "B" in phases:
                  nc.leave_named_scope("phB", _sidB, False)
              _sidC, _ = nc.enter_named_scope("phC", False)
              with (tc.tile_pool(name="wo", bufs=1) as wop,
                  tc.tile_pool(name="otm", bufs=3) as otmp,
                  tc.tile_pool(name="evC", bufs=3) as evc,
                  tc.tile_pool(name="psC", bufs=2, space="PSUM") as psC):
                n_mt = c.NTOK // 128
                half = c.hidden // 2          # columns per evacuation slab
                n_ct = half // c.tokt         # moving tiles per half
                wts = []
                for jd in range(c.HL):
                    wt = wop.tile([128, c.hidden], BF16, tag=f"wo{jd}")
                    nc.sync.dma_start(
                        wt[:], Wo.ap()[jd * 128:(jd + 1) * 128, :])
                    wts.append(wt)
                for m in range(n_mt):
                    # one DMA for all 4 jd chunks: [128, HL*128] bf16
                    om = otmp.tile([128, c.HL * 128], BF16, tag="otm")
                    nc.sync.dma_start(
                        om[:], OTd.ap()[:, m * 128:(m + 1) * 128]
                        .rearrange("(c p) n -> p c n", p=128))
                    for r in range(2):
                        pss = [psC.tile([128, c.tokt], F32, tag=f"c{ci}",
                                        name=f"psC_{m}_{r}_{ci}")
                               for ci in range(n_ct)]
                        # jd-outer: one LDWEIGHTS per jd per half
                        for jd in range(c.HL):
                            for ci in range(n_ct):
                                col = r * half + ci * c.tokt
                                nc.tensor.matmul(
                                    pss[ci][:],
                                    om[:, jd * 128:(jd + 1) * 128],
                                    wts[jd][:, col:col + c.tokt],
                                    start=(jd == 0), stop=(jd == c.HL - 1))
                        ev = evc.tile([128, half], BF16, tag="ev")
                        for ci in range(n_ct):
                            dst = ev[:, ci * c.tokt:(ci + 1) * c.tokt]
                            if ci % 2 == 0:
                                nc.vector.tensor_copy(dst, pss[ci][:])
                            else:
                                nc.scalar.copy(dst, pss[ci][:])
                        nc.sync.dma_start(
                            out.ap()[m * 128:(m + 1) * 128,
                                     r * half:(r + 1) * half], ev[:])
              nc.leave_named_scope("phC", _sidC, False)
    nc.compile()
    return nc


def rope_tables(positions, s_batch):
    import ml_dtypes
    bf = ml_dtypes.bfloat16
    pos = np.asarray(positions).astype(np.float64)
    inv = ROPE_BASE ** (-np.arange(0, HEAD_DIM, 2, dtype=np.float64) / HEAD_DIM)
    fr = pos[None, :] * inv[:, None]            # [64, S]
    cosT = np.concatenate([np.cos(fr), np.cos(fr)], 0).astype(bf)
    sinT = np.concatenate([-np.sin(fr), np.sin(fr)], 0).astype(bf)
    return np.ascontiguousarray(cosT), np.ascontiguousarray(sinT)


def make_in_maps(cfg, positions, hidden_states, Wq, Wk, Wv, Wo, n_cores=N_CORES):
    import ml_dtypes
    bf = ml_dtypes.bfloat16
    c = cfg
    hs = np.asarray(hidden_states, dtype=np.float32)
    hsT = np.ascontiguousarray(hs.reshape(c.NTOK, c.hidden).T.astype(bf))
    cosT, sinT = rope_tables(positions, c.s_batch)
    Wq = np.asarray(Wq, dtype=np.float32).astype(bf)
    Wk = np.asarray(Wk, dtype=np.float32).astype(bf)
    Wv = np.asarray(Wv, dtype=np.float32).astype(bf)
    Wo = np.asarray(Wo, dtype=np.float32).astype(bf)
    in_maps = []
    for r in range(n_cores):
        d0 = r * c.d_shard
        in_maps.append({
            "hsT": hsT,
            "Wq": np.ascontiguousarray(Wq[:, d0:d0 + c.d_shard]),
            "Wk": np.ascontiguousarray(Wk[:, d0:d0 + c.d_shard]),
            "Wv": np.ascontiguousarray(Wv[:, d0:d0 + c.d_shard]),
            "Wo": np.ascontiguousarray(Wo[d0:d0 + c.d_shard, :]),
            "cosT": cosT,
            "sinT": sinT,
        })
    return in_maps


_NC_CACHE = {}


def get_nc(cfg=None, n_cores=N_CORES):
    cfg = cfg or Cfg()
    key = (cfg.hidden, cfg.d_shard, cfg.s_batch, cfg.n_batch, n_cores)
    if key not in _NC_CACHE:
        _NC_CACHE[key] = (cfg, build_nc(cfg, n_cores))
    return _NC_CACHE[key]


def kernel(positions, hidden_states, Wq, Wk, Wv, Wo):
    cfg, nc = get_nc()
    in_maps = make_in_maps(cfg, positions, hidden_states, Wq, Wk, Wv, Wo)
    res = bass_utils.run_bass_kernel_spmd(nc, in_maps,
                                          core_ids=list(range(N_CORES)))
    acc = np.zeros((cfg.NTOK, cfg.hidden), dtype=np.float32)
    for r in res.results:
        acc += np.asarray(r["out"], dtype=np.float32)
    return acc.reshape(cfg.n_batch, cfg.s_batch, cfg.hidden)
